# revision 13
# baseline (speedup 1.0000x reference)
"""Coupled-attention module as a distributed Bass/Tile kernel on 8 TRN2 cores.

Math notes (exact algebra, not approximations):
- The differential-attention scores are constant along the softmax axis, so
  softmax yields exactly uniform 1/S weights: diff_vector collapses to the
  per-batch mean of (y @ dv_w + dv_b), broadcast over sequence. dq/dk are dead.
- Sharding: cores 0-3 own batch 0, cores 4-7 batch 1. Attention is
  tensor-parallel over heads (3 heads/core, all 1024 queries), so each core
  only computes 192 columns of the q/k/v projections. One AllToAll within
  each 4-core group reshards the attention output from head-major to
  row-major (256 rows/core) for the gating network.
- The two sequence-axis softmax denominators in the gating network are summed
  across the 4-core batch group with small AllGathers (cheaper floor than
  AllReduce) + local adds.
- Per-batch constants (m, th1, bias1, bias2) use vector-stationary matmuls
  (weights as the moving operand) with DRAM-bounce transposes, emitted inside
  the attention phase so they fill TensorE gaps while ACT computes exp.
- Compute in bf16 with fp32 accumulation; exp/tanh/sigmoid on ACT.
"""

import numpy as np
import ml_dtypes

import concourse.bass as bass
import concourse.mybir as mybir
import concourse.tile as tile
from concourse import bacc
from concourse.bass_utils import run_bass_kernel_spmd

B, S, H = 2, 1024, 768
NH, DH = 12, 64
P = 128
RV = 256            # gating rows per core
KC = H // P         # 6 channel chunks
JC = S // P         # 8 sequence chunks
HPC = 3             # heads per core
GROUPS = [[0, 1, 2, 3], [4, 5, 6, 7]]
SCALE = 1.0 / 8.0   # 1/sqrt(DH)

bf16 = mybir.dt.bfloat16
f32 = mybir.dt.float32
AF = mybir.ActivationFunctionType
ALU = mybir.AluOpType
nbf16 = ml_dtypes.bfloat16

# full weights every core loads
W768 = ["dv_w", "WD_w", "van_fc_w", "WV_w", "diff_fc_w",
        "diff_fus_w", "van_fus_w", "nf_w", "final_w"]
W1536 = ["d_theta_w", "v_gamma_w", "diff_out_w", "van_out_w"]
BIAS = ["dv_b", "van_fc_b", "d_theta_b", "diff_fc_b",
        "v_gamma_b", "diff_out_b", "van_out_b", "diff_fus_b", "van_fus_b",
        "nf_b", "final_b"]


def build(has_vvb: bool):
    nc = bacc.Bacc(None, target_bir_lowering=False, debug=False, num_devices=8)

    xT_d = nc.dram_tensor("xT", [H, S], bf16, kind="ExternalInput")
    yT_d = nc.dram_tensor("yT", [H, S], bf16, kind="ExternalInput")
    wq3_d = nc.dram_tensor("wq3", [H, HPC * DH], bf16, kind="ExternalInput")
    wk3_d = nc.dram_tensor("wk3", [H, HPC * DH], bf16, kind="ExternalInput")
    wv3_d = nc.dram_tensor("wv3", [H, HPC * DH], bf16, kind="ExternalInput")
    bq3_d = nc.dram_tensor("bq3", [HPC * DH], f32, kind="ExternalInput")
    bk3_d = nc.dram_tensor("bk3", [HPC * DH], f32, kind="ExternalInput")
    sel_d = nc.dram_tensor("sel", [2 * P], f32, kind="ExternalInput")
    if has_vvb:
        bv3_d = nc.dram_tensor("bv3", [HPC * DH], f32, kind="ExternalInput")
    wd = {}
    for w in W768:
        wd[w] = nc.dram_tensor(w, [H, H], bf16, kind="ExternalInput")
    for w in W1536:
        wd[w] = nc.dram_tensor(w, [2 * H, H], bf16, kind="ExternalInput")
    wd["gate_w"] = nc.dram_tensor("gate_w", [2 * H, 1], bf16, kind="ExternalInput")
    wd["nf_out_w"] = nc.dram_tensor("nf_out_w", [2 * H, 1], bf16, kind="ExternalInput")
    bd = {}
    for b in BIAS:
        bd[b] = nc.dram_tensor(b, [H], f32, kind="ExternalInput")
    out_d = nc.dram_tensor("outT", [H, RV], f32, kind="ExternalOutput")

    with tile.TileContext(nc, num_cores=8) as tc:
        with (
            tc.tile_pool(name="wpool", bufs=4) as wp,
            tc.tile_pool(name="wsmall", bufs=2) as wsp,
            tc.tile_pool(name="acts", bufs=1) as ap,
            tc.tile_pool(name="loop", bufs=2) as lp,
            tc.tile_pool(name="psum", bufs=8, space="PSUM") as pp,
            tc.tile_pool(name="dram", bufs=4, space="DRAM") as dp,
        ):
            # ---------------- input DMAs in consumption order ---------------
            wq3 = ap.tile([P, KC, HPC * DH], bf16, name="wq3")
            for kc in range(KC):
                nc.sync.dma_start(wq3[:, kc, :], wq3_d.rearrange(
                    "(kc p) n -> kc p n", p=P)[kc])
            xT = lp.tile([P, KC, S], bf16, name="xT", tag="expT", bufs=2)
            for kc in range(KC):
                nc.sync.dma_start(xT[:, kc, :], xT_d.rearrange(
                    "(kc p) n -> kc p n", p=P)[kc])
            wk3 = ap.tile([P, KC, HPC * DH], bf16, name="wk3")
            for kc in range(KC):
                nc.sync.dma_start(wk3[:, kc, :], wk3_d.rearrange(
                    "(kc p) n -> kc p n", p=P)[kc])
            yT = lp.tile([P, KC, S], bf16, name="yT", tag="expT", bufs=2)
            for kc in range(KC):
                nc.sync.dma_start(yT[:, kc, :], yT_d.rearrange(
                    "(kc p) n -> kc p n", p=P)[kc])
            wv3 = ap.tile([P, KC, HPC * DH], bf16, name="wv3")
            for kc in range(KC):
                nc.sync.dma_start(wv3[:, kc, :], wv3_d.rearrange(
                    "(kc p) n -> kc p n", p=P)[kc])
            bqp = ap.tile([P, 1], f32, name="bqp")
            nc.sync.dma_start(bqp[:], bq3_d[0:P].rearrange("(c p) -> p c", p=P))
            bq2 = ap.tile([64, 1], f32, name="bq2")
            nc.sync.dma_start(bq2[:], bq3_d[P:P + 64].rearrange("(c p) -> p c", p=64))
            bkp = ap.tile([P, 1], f32, name="bkp")
            nc.sync.dma_start(bkp[:], bk3_d[0:P].rearrange("(c p) -> p c", p=P))
            bk2 = ap.tile([64, 1], f32, name="bk2")
            nc.sync.dma_start(bk2[:], bk3_d[P:P + 64].rearrange("(c p) -> p c", p=64))

            selA = ap.tile([P, 1], f32, name="selA")
            nc.sync.dma_start(selA[:], sel_d[0:P].rearrange("(c p) -> p c", p=P))
            selB = ap.tile([P, 1], f32, name="selB")
            nc.sync.dma_start(selB[:], sel_d[P:2 * P].rearrange("(c p) -> p c", p=P))

            ones64 = ap.tile([1, 64], bf16, name="ones64")
            nc.vector.memset(ones64[:], 1.0)
            ones128 = ap.tile([1, P], f32, name="ones128")
            nc.vector.memset(ones128[:], 1.0)

            def wtile(name, half=None):
                t = wp.tile([P, KC, H], bf16, name=f"w_{name}_{half}", tag="w")
                src = wd[name]
                if half is not None:
                    src = src[half * H:(half + 1) * H, :]
                src = src.rearrange("(kc p) n -> kc p n", p=P)
                for kc in range(KC):
                    nc.sync.dma_start(t[:, kc, :], src[kc])
                return t

            def btile(name):
                t = ap.tile([P, KC], f32, name=f"b_{name}")
                nc.sync.dma_start(t[:], bd[name].rearrange("(c p) -> p c", p=P))
                return t

            def brow(name):
                # bias as a [1, 768] row vector (for row-major chain outputs)
                t = ap.tile([1, H], f32, name=f"br_{name}")
                nc.sync.dma_start(t[:], bd[name].rearrange("(o c) -> o c", o=1))
                return t

            # ---------------- Q/K projections (chan-major, head-sharded) ----
            # qTp/kTp: heads 0,1 stacked on partitions; qT2/kT2: head 2.
            def proj_cm(w_t, rhs_t, bias_p, bias_2, name):
                tp = ap.tile([P, S], bf16, name=f"{name}p")
                t2 = ap.tile([64, S], bf16, name=f"{name}2")
                for qh in range(2):
                    ps = pp.tile([P, 512], f32, name=f"{name}ps{qh}", tag="big",
                                 bufs=3)
                    for kc in range(KC):
                        nc.tensor.matmul(ps[:], w_t[:, kc, 0:P],
                                         rhs_t[:, kc, qh * 512:(qh + 1) * 512],
                                         start=(kc == 0), stop=(kc == KC - 1))
                    nc.vector.tensor_scalar_add(tp[:, qh * 512:(qh + 1) * 512],
                                                ps[:], bias_p[:, 0:1])
                for qh in range(2):
                    ps2 = pp.tile([64, 512], f32, name=f"{name}ps2{qh}",
                                  tag="big", bufs=3)
                    for kc in range(KC):
                        nc.tensor.matmul(ps2[:], w_t[:, kc, P:P + 64],
                                         rhs_t[:, kc, qh * 512:(qh + 1) * 512],
                                         start=(kc == 0), stop=(kc == KC - 1))
                    nc.vector.tensor_scalar_add(t2[:, qh * 512:(qh + 1) * 512],
                                                ps2[:], bias_2[:, 0:1])
                return tp, t2

            qTp, qT2 = proj_cm(wq3, xT, bqp, bq2, "qT")
            kTp, kT2 = proj_cm(wk3, yT, bkp, bk2, "kT")

            # ---------------- V projection (row-major + ones col) -----------
            v_aug = ap.tile([P, JC, HPC, DH + 1], bf16, name="v_aug")
            nc.vector.memset(v_aug[:, :, :, DH:DH + 1], 1.0)
            for jc in range(JC):
                ps = pp.tile([P, HPC * DH], f32, name=f"vps{jc}", tag="big",
                             bufs=3)
                for kc in range(KC):
                    nc.tensor.matmul(ps[:], yT[:, kc, jc * P:(jc + 1) * P],
                                     wv3[:, kc, :],
                                     start=(kc == 0), stop=(kc == KC - 1))
                nc.vector.tensor_copy(
                    v_aug[:, jc, :, 0:DH],
                    ps[:].rearrange("p (h d) -> p h d", d=DH))

            # ---------------- per-batch chain helpers -----------------------
            # ybar (mean of y over sequence), chan-major [128, 6]
            yb = ap.tile([P, KC], f32, name="yb")
            for kc in range(KC):
                nc.vector.tensor_reduce(yb[:, kc:kc + 1], yT[:, kc, :],
                                        axis=mybir.AxisListType.X, op=ALU.add)
            ybt = ap.tile([P, KC], bf16, name="ybt")
            nc.vector.tensor_scalar_mul(ybt[:], yb[:], 1.0 / S)

            def vchain(vec_cm, w_t, func, bias_row, name):
                # row-major out [1, 768] = func(vec @ W + bias); vec chan-major
                # [128, 6] bf16 is the stationary operand (weights stream).
                pr = []
                for half in range(2):
                    ps = pp.tile([1, 384], f32, name=f"{name}ps{half}",
                                 tag="sps", bufs=2)
                    for kc in range(KC):
                        nc.tensor.matmul(ps[:], vec_cm[:, kc:kc + 1],
                                         w_t[:, kc, half * 384:(half + 1) * 384],
                                         start=(kc == 0), stop=(kc == KC - 1))
                    pr.append(ps)
                out = ap.tile([1, H], f32, name=f"{name}_row")
                for half in range(2):
                    osl = out[:, half * 384:(half + 1) * 384]
                    bsl = (None if bias_row is None
                           else bias_row[:, half * 384:(half + 1) * 384])
                    if func == AF.Identity:
                        if bsl is not None:
                            nc.vector.tensor_add(osl, pr[half][:], bsl)
                        else:
                            nc.vector.tensor_copy(osl, pr[half][:])
                    else:
                        src = pr[half]
                        if bsl is not None:
                            tmp = lp.tile([1, 384], f32, name=f"{name}tmp{half}",
                                          tag="chtmp")
                            nc.vector.tensor_add(tmp[:], src[:], bsl)
                            src = tmp
                        nc.scalar.activation(osl, src[:], func)
                return out

            def to_chan(row_t, dt, name):
                # [1, 768] row vector -> chan-major [128, 6] via DRAM bounce
                db = dp.tile([H], dt, name=f"db_{name}")
                if dt == f32:
                    nc.sync.dma_start(db.rearrange("(o c) -> o c", o=1), row_t[:])
                else:
                    cast = lp.tile([1, H], dt, name=f"cast_{name}", tag="chcast")
                    nc.vector.tensor_copy(cast[:], row_t[:])
                    nc.sync.dma_start(db.rearrange("(o c) -> o c", o=1), cast[:])
                out = ap.tile([P, KC], dt, name=f"cm_{name}")
                nc.sync.dma_start(out[:], db.rearrange("(c p) -> p c", p=P))
                return out

            # ---------------- attention (3 heads, all 1024 queries) ---------
            # chains are emitted between heads to fill PE gaps during exp.
            w_dv = wtile("dv_w")
            b_dv_row = brow("dv_b")
            w_WD = wtile("WD_w")
            w_dth0 = wtile("d_theta_w", half=0)
            b_dth_row = brow("d_theta_b")
            w_dout0 = wtile("diff_out_w", half=0)
            b_dout_row = brow("diff_out_b")
            if has_vvb:
                bv3p = ap.tile([P, 1], f32, name="bv3p")
                nc.sync.dma_start(bv3p[:], bv3_d[0:P].rearrange("(c p) -> p c", p=P))
                bv32 = ap.tile([64, 1], f32, name="bv32")
                nc.sync.dma_start(bv32[:], bv3_d[P:P + 64].rearrange("(c p) -> p c", p=64))

            vanP = ap.tile([P, S], bf16, name="vanP")    # heads 0,1 chan-major
            vanP2 = ap.tile([64, S], bf16, name="vanP2")  # head 2

            def pair_attn():
                # scores + exp for heads 0,1 interleaved: the two 64-row
                # stationary tiles land in different PE row groups, so their
                # matmuls run concurrently in the array.
                e0 = lp.tile([P, JC, S], bf16, name="e0", tag="expT", bufs=2)
                e1 = lp.tile([P, JC, S], bf16, name="e1", tag="expT", bufs=2)
                for jc in range(JC):
                    for qh in range(2):
                        for lo, e in ((0, e0), (64, e1)):
                            sc = pp.tile([P, 512], f32,
                                         name=f"sc{lo}_{jc}_{qh}",
                                         tag="big", bufs=3)
                            nc.tensor.matmul(
                                sc[:], kTp[lo:lo + 64, jc * P:(jc + 1) * P],
                                qTp[lo:lo + 64, qh * 512:(qh + 1) * 512],
                                start=True, stop=True)
                            nc.scalar.activation(
                                e[:, jc, qh * 512:(qh + 1) * 512],
                                sc[:], AF.Exp, scale=SCALE)
                return e0, e1

            def head_attn2():
                # scores + exp for head 2 (64-row stationary, solo)
                e = lp.tile([P, JC, S], bf16, name="e2", tag="expT", bufs=2)
                for jc in range(JC):
                    for qh in range(2):
                        sc = pp.tile([P, 512], f32, name=f"sc2_{jc}_{qh}",
                                     tag="big", bufs=3)
                        nc.tensor.matmul(
                            sc[:], kT2[0:64, jc * P:(jc + 1) * P],
                            qT2[0:64, qh * 512:(qh + 1) * 512],
                            start=True, stop=True)
                        nc.scalar.activation(e[:, jc, qh * 512:(qh + 1) * 512],
                                             sc[:], AF.Exp, scale=SCALE)
                return e

            def head_pv(h, e):
                # P@V + denominator; writes normalized output to vanP/vanP2
                dst = vanP if h < 2 else vanP2
                lo = (h % 2) * 64 if h < 2 else 0
                for qh in range(2):
                    pv = pp.tile([DH + 1, 512], f32, name=f"pv{h}_{qh}",
                                 tag="pv", bufs=3)
                    for jc in range(JC):
                        nc.tensor.matmul(pv[:], v_aug[:, jc, h, :],
                                         e[:, jc, qh * 512:(qh + 1) * 512],
                                         start=(jc == 0), stop=(jc == JC - 1))
                    invZ = lp.tile([1, 512], f32, name=f"invZ{h}{qh}", tag="invZ")
                    nc.vector.reciprocal(invZ[:], pv[DH:DH + 1, :])
                    invZb = lp.tile([1, 512], bf16, name=f"invZb{h}{qh}",
                                    tag="invZb")
                    nc.vector.tensor_copy(invZb[:], invZ[:])
                    bc = pp.tile([64, 512], f32, name=f"bc{h}{qh}", tag="big",
                                 bufs=3)
                    nc.tensor.matmul(bc[:], ones64[:], invZb[:],
                                     start=True, stop=True)
                    bcs = lp.tile([64, 512], bf16, name=f"bcs{h}{qh}", tag="bcs")
                    nc.vector.tensor_copy(bcs[:], bc[:])
                    nc.vector.tensor_mul(dst[lo:lo + 64, qh * 512:(qh + 1) * 512],
                                         pv[0:DH, :], bcs[:])

            e0, e1h = pair_attn()
            # chain: m = ybar @ dv_w + dv_b  (diff vector)
            m_row = vchain(ybt, w_dv, AF.Identity, b_dv_row, "m")
            head_pv(0, e0)
            m32 = to_chan(m_row, f32, "m32")
            m_cm = ap.tile([P, KC], bf16, name="m_cm")
            nc.vector.tensor_copy(m_cm[:], m32[:])
            th1_row = vchain(m_cm, w_WD, AF.Tanh, None, "th1")
            head_pv(1, e1h)
            e2h = head_attn2()
            th1_cm = to_chan(th1_row, bf16, "th1")
            bias1_row = vchain(th1_cm, w_dth0, AF.Identity, b_dth_row, "bias1")
            bias2_row = vchain(m_cm, w_dout0, AF.Identity, b_dout_row, "bias2")
            head_pv(2, e2h)
            bias1 = to_chan(bias1_row, f32, "bias1")
            bias2 = to_chan(bias2_row, f32, "bias2")

            if has_vvb:
                nc.vector.tensor_scalar_add(vanP[:], vanP[:], bv3p[:, 0:1])
                nc.vector.tensor_scalar_add(vanP2[:], vanP2[:], bv32[:, 0:1])

            # ---------------- AllToAll: head-major -> row-major -------------
            # 4-rank A2A is unsupported (mesh needs >4 ranks), so run one
            # 8-rank A2A with each core duplicating its shards into both
            # groups' slots; the receiver picks its batch's half with a
            # host-fed 0/1 mask (keeps the program SPMD-uniform).
            ci = dp.tile([8 * HPC * DH, RV], bf16, name="ci_a2a")
            co = dp.tile([8 * HPC * DH, RV], bf16, name="co_a2a")
            for j in range(4):
                for gofs in (0, 768):
                    nc.sync.dma_start(ci[gofs + j * 192:gofs + j * 192 + P, :],
                                      vanP[:, j * RV:(j + 1) * RV])
                    nc.sync.dma_start(
                        ci[gofs + j * 192 + P:gofs + (j + 1) * 192, :],
                        vanP2[:, j * RV:(j + 1) * RV])
            nc.gpsimd.collective_compute(
                "AllToAll", ALU.bypass, replica_groups=[list(range(8))],
                ins=[ci[:]], outs=[co[:]])
            co0 = ap.tile([P, KC, RV], bf16, name="co0")
            co1 = ap.tile([P, KC, RV], bf16, name="co1")
            for kc in range(KC):
                nc.sync.dma_start(co0[:, kc, :], co[0:H].rearrange(
                    "(kc p) n -> kc p n", p=P)[kc])
                nc.sync.dma_start(co1[:, kc, :], co[H:2 * H].rearrange(
                    "(kc p) n -> kc p n", p=P)[kc])
            vanT = ap.tile([P, KC, RV], bf16, name="vanT")
            for kc in range(KC):
                nc.vector.tensor_scalar_mul(vanT[:, kc, :], co0[:, kc, :],
                                            selA[:, 0:1])
                nc.vector.scalar_tensor_tensor(
                    vanT[:, kc, :], co1[:, kc, :], selB[:, 0:1],
                    vanT[:, kc, :], op0=ALU.mult, op1=ALU.add)

            # ---------------- gating network --------------------------------
            def gemm(pairs, func, bias_t=None, accum_t=None, name="g",
                     out_dt=bf16, pre=None):
                out = ap.tile([P, KC, RV], out_dt, name=name)
                nmm = len(pairs) * KC
                for mc in range(KC):
                    ps = pp.tile([P, RV], f32, name=f"{name}ps{mc}", tag="big",
                                 bufs=3)
                    i = 0
                    for wt, at in pairs:
                        for kc in range(KC):
                            nc.tensor.matmul(ps[:],
                                             wt[:, kc, mc * P:(mc + 1) * P],
                                             at[:, kc, :],
                                             start=(i == 0), stop=(i == nmm - 1))
                            i += 1
                    src = ps
                    if pre is not None:
                        tmp = lp.tile([P, RV], f32, name=f"{name}pre{mc}",
                                      tag="pretmp")
                        nc.vector.tensor_add(tmp[:], ps[:], pre[:, mc, :])
                        src = tmp
                    if func == AF.Identity and accum_t is None:
                        if bias_t is not None:
                            nc.vector.tensor_scalar_add(out[:, mc, :], src[:],
                                                        bias_t[:, mc:mc + 1])
                        else:
                            nc.vector.tensor_copy(out[:, mc, :], src[:])
                    else:
                        nc.scalar.activation(
                            out[:, mc, :], src[:], func,
                            bias=(bias_t[:, mc:mc + 1] if bias_t is not None
                                  else 0.0),
                            accum_out=(accum_t[:, mc:mc + 1]
                                       if accum_t is not None else None))
                return out

            def allgather6(part, name):
                gi = dp.tile([P, KC], f32, name=f"gi_{name}")
                go = dp.tile([4 * P, KC], f32, name=f"go_{name}")
                nc.sync.dma_start(gi[:], part[:])
                nc.gpsimd.collective_compute(
                    "AllGather", ALU.bypass, replica_groups=GROUPS,
                    ins=[gi[:]], outs=[go[:]])
                zt = ap.tile([P, 4, KC], f32, name=f"zt_{name}")
                nc.sync.dma_start(zt[:], go.rearrange("(r p) c -> p r c", p=P))
                z = ap.tile([P, KC], f32, name=f"z_{name}")
                nc.vector.tensor_add(z[:], zt[:, 0, :], zt[:, 1, :])
                nc.vector.tensor_add(z[:], z[:], zt[:, 2, :])
                nc.vector.tensor_add(z[:], z[:], zt[:, 3, :])
                return z

            w_vfc = wtile("van_fc_w")
            b_vfc = btile("van_fc_b")
            theta2 = gemm([(w_vfc, vanT)], AF.Tanh, bias_t=b_vfc, name="theta2")

            w_dth1 = wtile("d_theta_w", half=1)
            part1 = ap.tile([P, KC], f32, name="part1")
            e1 = gemm([(w_dth1, theta2)], AF.Exp, bias_t=bias1, accum_t=part1,
                      name="e1")
            z1 = allgather6(part1, "z1")

            # --- AllGather-1 bubble fillers (independent of z1) -------------
            w_WV = wtile("WV_w")
            gamma1 = gemm([(w_WV, vanT)], AF.Tanh, name="gamma1")
            w_vg0 = wtile("v_gamma_w", half=0)
            b_vg = btile("v_gamma_b")
            z2a = gemm([(w_vg0, gamma1)], AF.Identity, bias_t=b_vg, name="z2a",
                       out_dt=f32)
            w_vo0 = wtile("van_out_w", half=0)
            b_vo = btile("van_out_b")
            voa = gemm([(w_vo0, vanT)], AF.Identity, bias_t=b_vo, name="voa",
                       out_dt=f32)

            s1 = ap.tile([P, KC], f32, name="s1")
            nc.vector.reciprocal(s1[:], z1[:])
            nc.vector.tensor_mul(s1[:], s1[:], m32[:])
            dth = ap.tile([P, KC, RV], bf16, name="dth")
            for mc in range(KC):
                nc.vector.tensor_scalar_mul(dth[:, mc, :], e1[:, mc, :],
                                            s1[:, mc:mc + 1])

            w_dfc = wtile("diff_fc_w")
            b_dfc = btile("diff_fc_b")
            gamma2 = gemm([(w_dfc, dth)], AF.Tanh, bias_t=b_dfc, name="gamma2")

            w_vg1 = wtile("v_gamma_w", half=1)
            part2 = ap.tile([P, KC], f32, name="part2")
            e2 = gemm([(w_vg1, gamma2)], AF.Exp, accum_t=part2, pre=z2a,
                      name="e2")
            z2 = allgather6(part2, "z2")

            # --- AllGather-2 bubble fillers (diff branch tail) --------------
            w_dout1 = wtile("diff_out_w", half=1)
            dout = gemm([(w_dout1, dth)], AF.Tanh, bias_t=bias2, name="dout")
            w_dfus = wtile("diff_fus_w")
            b_dfus = btile("diff_fus_b")
            dfus = gemm([(w_dfus, dout)], AF.Tanh, bias_t=b_dfus, name="dfus")

            s2 = ap.tile([P, KC], f32, name="s2")
            nc.vector.reciprocal(s2[:], z2[:])
            ag = ap.tile([P, KC, RV], bf16, name="ag")
            for mc in range(KC):
                nc.vector.scalar_tensor_tensor(
                    ag[:, mc, :], e2[:, mc, :], s2[:, mc:mc + 1],
                    vanT[:, mc, :], op0=ALU.mult, op1=ALU.mult)

            w_vo1 = wtile("van_out_w", half=1)
            vout = gemm([(w_vo1, ag)], AF.Tanh, pre=voa, name="vout")
            w_vfus = wtile("van_fus_w")
            b_vfus = btile("van_fus_b")
            vfus = gemm([(w_vfus, vout)], AF.Tanh, bias_t=b_vfus, name="vfus")

            # gate (M=1 GEMM over both fusion tensors)
            def vec_unit(wname, act_pairs, name):
                wt = wsp.tile([P, 2 * KC, 1], bf16, name=f"ws_{name}", tag="ws")
                nc.sync.dma_start(wt[:], wd[wname].rearrange(
                    "(c p) o -> p c o", p=P))
                ps = pp.tile([1, RV], f32, name=f"{name}ps", tag="sps", bufs=2)
                i = 0
                for at, base in act_pairs:
                    for kc in range(KC):
                        nc.tensor.matmul(ps[:], wt[:, base + kc, :],
                                         at[:, kc, :],
                                         start=(i == 0), stop=(i == 2 * KC - 1))
                        i += 1
                out = ap.tile([1, RV], f32, name=f"v_{name}")
                nc.scalar.activation(out[:], ps[:], AF.Sigmoid)
                return out

            g = vec_unit("gate_w", [(dfus, 0), (vfus, KC)], "gate")
            gbc = pp.tile([P, RV], f32, name="gbc", tag="pv", bufs=3)
            nc.tensor.matmul(gbc[:], ones128[:], g[:], start=True, stop=True)

            fus = ap.tile([P, KC, RV], bf16, name="fus")
            for mc in range(KC):
                t1 = lp.tile([P, RV], bf16, name=f"ft1_{mc}", tag="ft1")
                nc.vector.tensor_sub(t1[:], vfus[:, mc, :], dfus[:, mc, :])
                t2 = lp.tile([P, RV], bf16, name=f"ft2_{mc}", tag="ft2")
                nc.vector.tensor_mul(t2[:], t1[:], gbc[:])
                nc.vector.tensor_add(fus[:, mc, :], t2[:], dfus[:, mc, :])

            w_nf = wtile("nf_w")
            b_nf = btile("nf_b")
            tnf = gemm([(w_nf, fus)], AF.Identity, bias_t=b_nf, name="tnf")
            nfv = vec_unit("nf_out_w", [(vanT, 0), (tnf, KC)], "nf")
            nbc = pp.tile([P, RV], f32, name="nbc", tag="pv", bufs=3)
            nc.tensor.matmul(nbc[:], ones128[:], nfv[:], start=True, stop=True)

            w_fin = wtile("final_w")
            b_fin = btile("final_b")
            ft = gemm([(w_fin, fus)], AF.Tanh, bias_t=b_fin, name="ftanh")
            od = out_d.rearrange("(mc p) n -> mc p n", p=P)
            for mc in range(KC):
                ot = lp.tile([P, RV], f32, name=f"ot{mc}", tag="ot")
                nc.vector.tensor_mul(ot[:], ft[:, mc, :], nbc[:])
                nc.sync.dma_start(od[mc], ot[:])

    nc.compile()
    return nc


_CACHE = {}


def _prep_in_maps(inputs):
    x = np.asarray(inputs["x"], np.float32)
    y = np.asarray(inputs["y"], np.float32)
    has_vvb = bool(np.any(np.asarray(inputs["vv_b"]) != 0))

    xts = [np.ascontiguousarray(x[b].T).astype(nbf16) for b in range(B)]
    yts = [np.ascontiguousarray(y[b].T).astype(nbf16) for b in range(B)]

    base = {}
    for w in W768 + W1536 + ["gate_w", "nf_out_w"]:
        base[w] = np.asarray(inputs[w], np.float32).astype(nbf16)
    for b in BIAS:
        base[b] = np.ascontiguousarray(np.asarray(inputs[b], np.float32))

    vq_w = np.asarray(inputs["vq_w"], np.float32)
    vk_w = np.asarray(inputs["vk_w"], np.float32)
    vv_w = np.asarray(inputs["vv_w"], np.float32)
    vq_b = np.asarray(inputs["vq_b"], np.float32)
    vk_b = np.asarray(inputs["vk_b"], np.float32)
    vv_b = np.asarray(inputs["vv_b"], np.float32)

    in_maps = []
    for c in range(8):
        bat, loc = c // 4, c % 4
        sl = slice(loc * HPC * DH, (loc + 1) * HPC * DH)
        m = dict(base)
        m["xT"] = xts[bat]
        m["yT"] = yts[bat]
        m["wq3"] = np.ascontiguousarray(vq_w[:, sl]).astype(nbf16)
        m["wk3"] = np.ascontiguousarray(vk_w[:, sl]).astype(nbf16)
        m["wv3"] = np.ascontiguousarray(vv_w[:, sl]).astype(nbf16)
        m["bq3"] = np.ascontiguousarray(vq_b[sl])
        m["bk3"] = np.ascontiguousarray(vk_b[sl])
        sel = np.zeros(2 * P, np.float32)
        sel[bat * P:(bat + 1) * P] = 1.0
        m["sel"] = sel
        if has_vvb:
            m["bv3"] = np.ascontiguousarray(vv_b[sl])
        in_maps.append(m)
    return in_maps, has_vvb


def kernel(**inputs):
    in_maps, has_vvb = _prep_in_maps(inputs)
    if has_vvb not in _CACHE:
        _CACHE[has_vvb] = build(has_vvb)
    nc = _CACHE[has_vvb]

    res = run_bass_kernel_spmd(nc, in_maps, core_ids=list(range(8)))
    full = np.concatenate([res.results[c]["outT"] for c in range(8)], axis=1)
    return np.ascontiguousarray(full.T.reshape(B, S, H)).astype(np.float32)


if __name__ == "__main__":
    rng = np.random.default_rng(0)
    ins = {"x": rng.standard_normal((B, S, H)).astype(np.float32),
           "y": rng.standard_normal((B, S, H)).astype(np.float32)}
    for w in ["vq_w", "vk_w", "vv_w", "dq_w", "dk_w"] + W768 + W1536:
        shp = (H, H) if w not in W1536 else (2 * H, H)
        ins[w] = (rng.standard_normal(shp) * 0.02).astype(np.float32)
    ins["gate_w"] = (rng.standard_normal((2 * H, 1)) * 0.02).astype(np.float32)
    ins["nf_out_w"] = (rng.standard_normal((2 * H, 1)) * 0.02).astype(np.float32)
    for b in BIAS + ["vq_b", "vk_b", "vv_b", "dq_b", "dk_b"]:
        ins[b] = np.zeros(H, np.float32)
    out = kernel(**ins)
    print("out", out.shape, out.dtype, np.abs(out).mean())


# revision 16
# speedup vs baseline: 1.1530x; 1.1530x over previous
"""Coupled-attention module as a distributed Bass/Tile kernel on 8 TRN2 cores.

Math notes (exact algebra, not approximations):
- The differential-attention scores are constant along the softmax axis, so
  softmax yields exactly uniform 1/S weights: diff_vector collapses to the
  per-batch mean of (y @ dv_w + dv_b), broadcast over sequence. dq/dk are dead.
- Sharding: rows of the flattened (B*S, H) activations, 256 per core; cores
  0-3 own batch 0, 4-7 batch 1. Each core redundantly computes full-batch K/V
  (collective reshards measure slower than the redundant GEMMs on this part).
- Attention head pairs are packed into disjoint PE row groups (K=64 each), so
  the two scores matmuls of a pair run concurrently, and both heads' scores
  share one [128,512] PSUM bank -> one big exp ACTIVATE per (pair, block).
- The two sequence-axis softmax denominators in the gating network are summed
  across the 4-core batch group with small AllGathers + local adds (lower
  floor than AllReduce).
- Per-batch constants (m, th1, bias1, bias2) use vector-stationary matmuls
  (weights as the moving operand) with DRAM-bounce transposes, emitted inside
  the attention phase so they fill TensorE gaps while ACT computes exp.
- Compute in bf16 with fp32 accumulation; exp/tanh/sigmoid on ACT; identity
  epilogues on DVE to keep ACT for transcendentals.
"""

import numpy as np
import ml_dtypes

import concourse.bass as bass
import concourse.mybir as mybir
import concourse.tile as tile
from concourse import bacc
from concourse.bass_utils import run_bass_kernel_spmd

B, S, H = 2, 1024, 768
NH, DH = 12, 64
P = 128
RV = 256            # rows per core
KC = H // P         # 6 channel chunks
JC = S // P         # 8 sequence chunks
GROUPS = [[0, 1, 2, 3], [4, 5, 6, 7]]
SCALE = 1.0 / 8.0   # 1/sqrt(DH)

bf16 = mybir.dt.bfloat16
f32 = mybir.dt.float32
AF = mybir.ActivationFunctionType
ALU = mybir.AluOpType
nbf16 = ml_dtypes.bfloat16

W768 = ["vq_w", "vk_w", "vv_w", "dv_w", "WD_w", "van_fc_w", "WV_w", "diff_fc_w",
        "diff_fus_w", "van_fus_w", "nf_w", "final_w"]
W1536 = ["d_theta_w", "v_gamma_w", "diff_out_w", "van_out_w"]
BIAS = ["vq_b", "vk_b", "dv_b", "van_fc_b", "d_theta_b", "diff_fc_b",
        "v_gamma_b", "diff_out_b", "van_out_b", "diff_fus_b", "van_fus_b",
        "nf_b", "final_b"]


def build(has_vvb: bool):
    nc = bacc.Bacc(None, target_bir_lowering=False, debug=False, num_devices=8)

    xT_d = nc.dram_tensor("xT", [H, RV], bf16, kind="ExternalInput")
    yT_d = nc.dram_tensor("yT", [H, S], bf16, kind="ExternalInput")
    wd = {}
    for w in W768:
        wd[w] = nc.dram_tensor(w, [H, H], bf16, kind="ExternalInput")
    for w in W1536:
        wd[w] = nc.dram_tensor(w, [2 * H, H], bf16, kind="ExternalInput")
    wd["gate_w"] = nc.dram_tensor("gate_w", [2 * H, 1], bf16, kind="ExternalInput")
    wd["nf_out_w"] = nc.dram_tensor("nf_out_w", [2 * H, 1], bf16, kind="ExternalInput")
    bd = {}
    for b in BIAS:
        bd[b] = nc.dram_tensor(b, [H], f32, kind="ExternalInput")
    if has_vvb:
        bd["vv_b"] = nc.dram_tensor("vv_b", [H], f32, kind="ExternalInput")
    out_d = nc.dram_tensor("outT", [H, RV], f32, kind="ExternalOutput")

    with tile.TileContext(nc, num_cores=8) as tc:
        with (
            tc.tile_pool(name="wpool", bufs=4) as wp,
            tc.tile_pool(name="wsmall", bufs=2) as wsp,
            tc.tile_pool(name="acts", bufs=1) as ap,
            tc.tile_pool(name="loop", bufs=2) as lp,
            tc.tile_pool(name="psum", bufs=8, space="PSUM") as pp,
            tc.tile_pool(name="dram", bufs=4, space="DRAM") as dp,
        ):
            def wtile(name, half=None):
                t = wp.tile([P, KC, H], bf16, name=f"w_{name}_{half}", tag="w")
                src = wd[name]
                if half is not None:
                    src = src[half * H:(half + 1) * H, :]
                src = src.rearrange("(kc p) n -> kc p n", p=P)
                for kc in range(KC):
                    nc.sync.dma_start(t[:, kc, :], src[kc])
                return t

            def btile(name):
                t = ap.tile([P, KC], f32, name=f"b_{name}")
                nc.sync.dma_start(t[:], bd[name].rearrange("(c p) -> p c", p=P))
                return t

            def brow(name):
                t = ap.tile([1, H], f32, name=f"br_{name}")
                nc.sync.dma_start(t[:], bd[name].rearrange("(o c) -> o c", o=1))
                return t

            # ---------------- Q projection first: minimal-dependency PE work
            b_vq = btile("vq_b")
            xT = ap.tile([P, KC, RV], bf16, name="xT")
            for kc in range(KC):
                nc.sync.dma_start(xT[:, kc, :], xT_d.rearrange(
                    "(kc p) n -> kc p n", p=P)[kc])
            w_vq = wtile("vq_w")
            qT = ap.tile([P, KC, RV], bf16, name="qT")
            for mc in range(KC):
                ps = pp.tile([P, RV], f32, name=f"qps{mc}", tag="big", bufs=3)
                for kc in range(KC):
                    nc.tensor.matmul(ps[:], w_vq[:, kc, mc * P:(mc + 1) * P],
                                     xT[:, kc, :],
                                     start=(kc == 0), stop=(kc == KC - 1))
                nc.vector.tensor_scalar_add(qT[:, mc, :], ps[:],
                                            b_vq[:, mc:mc + 1])

            b_vk = btile("vk_b")
            yT = ap.tile([P, KC, S], bf16, name="yT")
            for kc in range(KC):
                nc.sync.dma_start(yT[:, kc, :], yT_d.rearrange(
                    "(kc p) n -> kc p n", p=P)[kc])

            ones64 = ap.tile([1, 64], bf16, name="ones64")
            nc.vector.memset(ones64[:], 1.0)
            ones128 = ap.tile([1, P], f32, name="ones128")
            nc.vector.memset(ones128[:], 1.0)

            # ---------------- K projection (full batch, channel-major) ------
            w_vk = wtile("vk_w")
            kT = ap.tile([P, KC, S], bf16, name="kT")
            for mc in range(KC):
                for sh in range(2):
                    ps = pp.tile([P, 512], f32, name=f"kps{mc}_{sh}",
                                 tag="big", bufs=3)
                    for kc in range(KC):
                        nc.tensor.matmul(
                            ps[:], w_vk[:, kc, mc * P:(mc + 1) * P],
                            yT[:, kc, sh * 512:(sh + 1) * 512],
                            start=(kc == 0), stop=(kc == KC - 1))
                    nc.vector.tensor_scalar_add(
                        kT[:, mc, sh * 512:(sh + 1) * 512], ps[:],
                        b_vk[:, mc:mc + 1])

            # ---------------- V projection (row-major + ones col) -----------
            w_vv = wtile("vv_w")
            v_aug = ap.tile([P, JC, NH, DH + 1], bf16, name="v_aug")
            nc.vector.memset(v_aug[:, :, :, DH:DH + 1], 1.0)
            for jc in range(JC):
                for cg in range(2):
                    ps = pp.tile([P, 384], f32, name=f"vps{jc}_{cg}",
                                 tag="big", bufs=3)
                    for kc in range(KC):
                        nc.tensor.matmul(
                            ps[:], yT[:, kc, jc * P:(jc + 1) * P],
                            w_vv[:, kc, cg * 384:(cg + 1) * 384],
                            start=(kc == 0), stop=(kc == KC - 1))
                    nc.vector.tensor_copy(
                        v_aug[:, jc, cg * 6:(cg + 1) * 6, 0:DH],
                        ps[:].rearrange("p (h d) -> p h d", d=DH))

            # ---------------- per-batch chain pieces ------------------------
            yb = ap.tile([P, KC], f32, name="yb")
            for kc in range(KC):
                nc.vector.tensor_reduce(yb[:, kc:kc + 1], yT[:, kc, :],
                                        axis=mybir.AxisListType.X, op=ALU.add)
            ybt = ap.tile([P, KC], bf16, name="ybt")
            nc.vector.tensor_scalar_mul(ybt[:], yb[:], 1.0 / S)

            def vchain(vec_cm, w_t, func, bias_row, name):
                # row-major out [1, 768] = func(vec @ W + bias); vec chan-major
                # [128, 6] bf16 is the stationary operand (weights stream).
                pr = []
                for half in range(2):
                    ps = pp.tile([1, 384], f32, name=f"{name}ps{half}",
                                 tag="sps", bufs=2)
                    for kc in range(KC):
                        nc.tensor.matmul(ps[:], vec_cm[:, kc:kc + 1],
                                         w_t[:, kc, half * 384:(half + 1) * 384],
                                         start=(kc == 0), stop=(kc == KC - 1))
                    pr.append(ps)
                out = ap.tile([1, H], f32, name=f"{name}_row")
                for half in range(2):
                    osl = out[:, half * 384:(half + 1) * 384]
                    bsl = (None if bias_row is None
                           else bias_row[:, half * 384:(half + 1) * 384])
                    if func == AF.Identity:
                        if bsl is not None:
                            nc.vector.tensor_add(osl, pr[half][:], bsl)
                        else:
                            nc.vector.tensor_copy(osl, pr[half][:])
                    else:
                        src = pr[half]
                        if bsl is not None:
                            tmp = lp.tile([1, 384], f32, name=f"{name}tmp{half}",
                                          tag="chtmp")
                            nc.vector.tensor_add(tmp[:], src[:], bsl)
                            src = tmp
                        nc.scalar.activation(osl, src[:], func)
                return out

            def to_chan(row_t, dt, name):
                # [1, 768] row vector -> chan-major [128, 6] via DRAM bounce
                db = dp.tile([H], dt, name=f"db_{name}")
                if dt == f32:
                    nc.sync.dma_start(db.rearrange("(o c) -> o c", o=1), row_t[:])
                else:
                    cast = lp.tile([1, H], dt, name=f"cast_{name}", tag="chcast")
                    nc.vector.tensor_copy(cast[:], row_t[:])
                    nc.sync.dma_start(db.rearrange("(o c) -> o c", o=1), cast[:])
                out = ap.tile([P, KC], dt, name=f"cm_{name}")
                nc.sync.dma_start(out[:], db.rearrange("(c p) -> p c", p=P))
                return out

            # ---------------- attention (6 head pairs, 256 own queries) -----
            w_dv = wtile("dv_w")
            b_dv_row = brow("dv_b")
            w_WD = wtile("WD_w")
            w_dth0 = wtile("d_theta_w", half=0)
            b_dth_row = brow("d_theta_b")
            w_dout0 = wtile("diff_out_w", half=0)
            b_dout_row = brow("diff_out_b")
            if has_vvb:
                b_vv = btile("vv_b")
            vanT = ap.tile([P, KC, RV], bf16, name="vanT")

            def pair_scores(hp):
                # the pair's two matmuls live in disjoint PE row groups ->
                # concurrent on the array
                e = lp.tile([P, JC, 512], bf16, name=f"e{hp}", tag="expT",
                            bufs=3)
                for jc in range(JC):
                    scs = []
                    for hh in range(2):
                        lo = hh * 64
                        sc = pp.tile([P, RV], f32, name=f"sc{hp}_{jc}_{hh}",
                                     tag="big", bufs=3)
                        nc.tensor.matmul(
                            sc[:],
                            kT[lo:lo + 64, hp, jc * P:(jc + 1) * P],
                            qT[lo:lo + 64, hp, :],
                            start=True, stop=True)
                        scs.append(sc)
                    for hh in range(2):
                        nc.scalar.activation(
                            e[:, jc, hh * RV:(hh + 1) * RV], scs[hh][:],
                            AF.Exp, scale=SCALE)
                return e

            def pair_pv(hp, e):
                for hh in range(2):
                    h = 2 * hp + hh
                    pv = pp.tile([DH + 1, RV], f32, name=f"pv{h}", tag="pv",
                                 bufs=3)
                    for jc in range(JC):
                        nc.tensor.matmul(pv[:], v_aug[:, jc, h, :],
                                         e[:, jc, hh * RV:(hh + 1) * RV],
                                         start=(jc == 0), stop=(jc == JC - 1))
                    invZ = lp.tile([1, RV], f32, name=f"invZ{h}", tag="invZ")
                    nc.vector.reciprocal(invZ[:], pv[DH:DH + 1, :])
                    invZb = lp.tile([1, RV], bf16, name=f"invZb{h}", tag="invZb")
                    nc.vector.tensor_copy(invZb[:], invZ[:])
                    bc = pp.tile([64, RV], f32, name=f"bc{h}", tag="big",
                                 bufs=3)
                    nc.tensor.matmul(bc[:], ones64[:], invZb[:],
                                     start=True, stop=True)
                    bcs = lp.tile([64, RV], bf16, name=f"bcs{h}", tag="bcs")
                    nc.vector.tensor_copy(bcs[:], bc[:])
                    nc.vector.tensor_mul(vanT[hh * 64:hh * 64 + 64, hp, :],
                                         pv[0:DH, :], bcs[:])
                    if has_vvb:
                        nc.vector.tensor_scalar_add(
                            vanT[hh * 64:hh * 64 + 64, hp, :],
                            vanT[hh * 64:hh * 64 + 64, hp, :],
                            b_vv[hh * 64:hh * 64 + 64, hp:hp + 1])

            # interleave chains with attention pairs: chains fill PE gaps
            # while ACT runs exp, and their results are needed only in gating.
            e0 = pair_scores(0)
            m_row = vchain(ybt, w_dv, AF.Identity, b_dv_row, "m")
            pair_pv(0, e0)
            e1h = pair_scores(1)
            m32 = to_chan(m_row, f32, "m32")
            m_cm = ap.tile([P, KC], bf16, name="m_cm")
            nc.vector.tensor_copy(m_cm[:], m32[:])
            pair_pv(1, e1h)
            e2h = pair_scores(2)
            th1_row = vchain(m_cm, w_WD, AF.Tanh, None, "th1")
            pair_pv(2, e2h)
            e3h = pair_scores(3)
            th1_cm = to_chan(th1_row, bf16, "th1")
            bias1_row = vchain(th1_cm, w_dth0, AF.Identity, b_dth_row, "bias1")
            pair_pv(3, e3h)
            e4h = pair_scores(4)
            bias2_row = vchain(m_cm, w_dout0, AF.Identity, b_dout_row, "bias2")
            pair_pv(4, e4h)
            e5h = pair_scores(5)
            bias1 = to_chan(bias1_row, f32, "bias1")
            bias2 = to_chan(bias2_row, f32, "bias2")
            pair_pv(5, e5h)

            # ---------------- gating network --------------------------------
            def gemm(pairs, func, bias_t=None, accum_t=None, name="g",
                     out_dt=bf16, pre=None):
                out = ap.tile([P, KC, RV], out_dt, name=name)
                nmm = len(pairs) * KC
                for mc in range(KC):
                    ps = pp.tile([P, RV], f32, name=f"{name}ps{mc}", tag="big",
                                 bufs=3)
                    i = 0
                    for wt, at in pairs:
                        for kc in range(KC):
                            nc.tensor.matmul(ps[:],
                                             wt[:, kc, mc * P:(mc + 1) * P],
                                             at[:, kc, :],
                                             start=(i == 0), stop=(i == nmm - 1))
                            i += 1
                    src = ps
                    if pre is not None:
                        tmp = lp.tile([P, RV], f32, name=f"{name}pre{mc}",
                                      tag="pretmp")
                        nc.vector.tensor_add(tmp[:], ps[:], pre[:, mc, :])
                        src = tmp
                    if func == AF.Identity and accum_t is None:
                        if bias_t is not None:
                            nc.vector.tensor_scalar_add(out[:, mc, :], src[:],
                                                        bias_t[:, mc:mc + 1])
                        else:
                            nc.vector.tensor_copy(out[:, mc, :], src[:])
                    else:
                        nc.scalar.activation(
                            out[:, mc, :], src[:], func,
                            bias=(bias_t[:, mc:mc + 1] if bias_t is not None
                                  else 0.0),
                            accum_out=(accum_t[:, mc:mc + 1]
                                       if accum_t is not None else None))
                return out

            def allgather6(part, name):
                gi = dp.tile([P, KC], f32, name=f"gi_{name}")
                go = dp.tile([4 * P, KC], f32, name=f"go_{name}")
                nc.sync.dma_start(gi[:], part[:])
                nc.gpsimd.collective_compute(
                    "AllGather", ALU.bypass, replica_groups=GROUPS,
                    ins=[gi[:]], outs=[go[:]])
                zt = ap.tile([P, 4, KC], f32, name=f"zt_{name}")
                nc.sync.dma_start(zt[:], go.rearrange("(r p) c -> p r c", p=P))
                z = ap.tile([P, KC], f32, name=f"z_{name}")
                nc.vector.tensor_add(z[:], zt[:, 0, :], zt[:, 1, :])
                nc.vector.tensor_add(z[:], z[:], zt[:, 2, :])
                nc.vector.tensor_add(z[:], z[:], zt[:, 3, :])
                return z

            w_vfc = wtile("van_fc_w")
            b_vfc = btile("van_fc_b")
            theta2 = gemm([(w_vfc, vanT)], AF.Tanh, bias_t=b_vfc, name="theta2")

            w_dth1 = wtile("d_theta_w", half=1)
            part1 = ap.tile([P, KC], f32, name="part1")
            e1 = gemm([(w_dth1, theta2)], AF.Exp, bias_t=bias1, accum_t=part1,
                      name="e1")
            z1 = allgather6(part1, "z1")

            # --- AllGather-1 bubble fillers (independent of z1) -------------
            w_WV = wtile("WV_w")
            gamma1 = gemm([(w_WV, vanT)], AF.Tanh, name="gamma1")
            w_vg0 = wtile("v_gamma_w", half=0)
            b_vg = btile("v_gamma_b")
            z2a = gemm([(w_vg0, gamma1)], AF.Identity, bias_t=b_vg, name="z2a",
                       out_dt=f32)
            w_vo0 = wtile("van_out_w", half=0)
            b_vo = btile("van_out_b")
            voa = gemm([(w_vo0, vanT)], AF.Identity, bias_t=b_vo, name="voa",
                       out_dt=f32)

            s1 = ap.tile([P, KC], f32, name="s1")
            nc.vector.reciprocal(s1[:], z1[:])
            nc.vector.tensor_mul(s1[:], s1[:], m32[:])
            dth = ap.tile([P, KC, RV], bf16, name="dth")
            for mc in range(KC):
                nc.vector.tensor_scalar_mul(dth[:, mc, :], e1[:, mc, :],
                                            s1[:, mc:mc + 1])

            w_dfc = wtile("diff_fc_w")
            b_dfc = btile("diff_fc_b")
            gamma2 = gemm([(w_dfc, dth)], AF.Tanh, bias_t=b_dfc, name="gamma2")

            w_vg1 = wtile("v_gamma_w", half=1)
            part2 = ap.tile([P, KC], f32, name="part2")
            e2 = gemm([(w_vg1, gamma2)], AF.Exp, accum_t=part2, pre=z2a,
                      name="e2")
            z2 = allgather6(part2, "z2")

            # --- AllGather-2 bubble fillers (diff branch tail) --------------
            w_dout1 = wtile("diff_out_w", half=1)
            dout = gemm([(w_dout1, dth)], AF.Tanh, bias_t=bias2, name="dout")
            w_dfus = wtile("diff_fus_w")
            b_dfus = btile("diff_fus_b")
            dfus = gemm([(w_dfus, dout)], AF.Tanh, bias_t=b_dfus, name="dfus")

            s2 = ap.tile([P, KC], f32, name="s2")
            nc.vector.reciprocal(s2[:], z2[:])
            ag = ap.tile([P, KC, RV], bf16, name="ag")
            for mc in range(KC):
                nc.vector.scalar_tensor_tensor(
                    ag[:, mc, :], e2[:, mc, :], s2[:, mc:mc + 1],
                    vanT[:, mc, :], op0=ALU.mult, op1=ALU.mult)

            w_vo1 = wtile("van_out_w", half=1)
            vout = gemm([(w_vo1, ag)], AF.Tanh, pre=voa, name="vout")
            w_vfus = wtile("van_fus_w")
            b_vfus = btile("van_fus_b")
            vfus = gemm([(w_vfus, vout)], AF.Tanh, bias_t=b_vfus, name="vfus")

            # gate (M=1 GEMM over both fusion tensors)
            def vec_unit(wname, act_pairs, name):
                wt = wsp.tile([P, 2 * KC, 1], bf16, name=f"ws_{name}", tag="ws")
                nc.sync.dma_start(wt[:], wd[wname].rearrange(
                    "(c p) o -> p c o", p=P))
                ps = pp.tile([1, RV], f32, name=f"{name}ps", tag="sps", bufs=2)
                i = 0
                for at, base in act_pairs:
                    for kc in range(KC):
                        nc.tensor.matmul(ps[:], wt[:, base + kc, :],
                                         at[:, kc, :],
                                         start=(i == 0), stop=(i == 2 * KC - 1))
                        i += 1
                out = ap.tile([1, RV], f32, name=f"v_{name}")
                nc.scalar.activation(out[:], ps[:], AF.Sigmoid)
                return out

            g = vec_unit("gate_w", [(dfus, 0), (vfus, KC)], "gate")
            gbc = pp.tile([P, RV], f32, name="gbc", tag="pv", bufs=3)
            nc.tensor.matmul(gbc[:], ones128[:], g[:], start=True, stop=True)

            fus = ap.tile([P, KC, RV], bf16, name="fus")
            for mc in range(KC):
                t1 = lp.tile([P, RV], bf16, name=f"ft1_{mc}", tag="ft1")
                nc.vector.tensor_sub(t1[:], vfus[:, mc, :], dfus[:, mc, :])
                t2 = lp.tile([P, RV], bf16, name=f"ft2_{mc}", tag="ft2")
                nc.vector.tensor_mul(t2[:], t1[:], gbc[:])
                nc.vector.tensor_add(fus[:, mc, :], t2[:], dfus[:, mc, :])

            w_nf = wtile("nf_w")
            b_nf = btile("nf_b")
            tnf = gemm([(w_nf, fus)], AF.Identity, bias_t=b_nf, name="tnf")
            nfv = vec_unit("nf_out_w", [(vanT, 0), (tnf, KC)], "nf")
            nbc = pp.tile([P, RV], f32, name="nbc", tag="pv", bufs=3)
            nc.tensor.matmul(nbc[:], ones128[:], nfv[:], start=True, stop=True)

            w_fin = wtile("final_w")
            b_fin = btile("final_b")
            ft = gemm([(w_fin, fus)], AF.Tanh, bias_t=b_fin, name="ftanh")
            od = out_d.rearrange("(mc p) n -> mc p n", p=P)
            for mc in range(KC):
                ot = lp.tile([P, RV], f32, name=f"ot{mc}", tag="ot")
                nc.vector.tensor_mul(ot[:], ft[:, mc, :], nbc[:])
                nc.sync.dma_start(od[mc], ot[:])

    nc.compile()
    return nc


_CACHE = {}


def _prep_in_maps(inputs):
    x = np.asarray(inputs["x"], np.float32)
    y = np.asarray(inputs["y"], np.float32)
    has_vvb = bool(np.any(np.asarray(inputs["vv_b"]) != 0))

    xt = np.ascontiguousarray(x.reshape(B * S, H).T).astype(nbf16)   # [H, 2048]
    yts = [np.ascontiguousarray(y[b].T).astype(nbf16) for b in range(B)]

    base = {}
    for w in W768 + W1536 + ["gate_w", "nf_out_w"]:
        base[w] = np.asarray(inputs[w], np.float32).astype(nbf16)
    for b in BIAS:
        base[b] = np.ascontiguousarray(np.asarray(inputs[b], np.float32))
    if has_vvb:
        base["vv_b"] = np.ascontiguousarray(np.asarray(inputs["vv_b"], np.float32))

    in_maps = []
    for c in range(8):
        bat = c // 4
        m = dict(base)
        m["xT"] = np.ascontiguousarray(xt[:, c * RV:(c + 1) * RV])
        m["yT"] = yts[bat]
        in_maps.append(m)
    return in_maps, has_vvb


def kernel(**inputs):
    in_maps, has_vvb = _prep_in_maps(inputs)
    if has_vvb not in _CACHE:
        _CACHE[has_vvb] = build(has_vvb)
    nc = _CACHE[has_vvb]

    res = run_bass_kernel_spmd(nc, in_maps, core_ids=list(range(8)))
    full = np.concatenate([res.results[c]["outT"] for c in range(8)], axis=1)
    return np.ascontiguousarray(full.T.reshape(B, S, H)).astype(np.float32)


if __name__ == "__main__":
    rng = np.random.default_rng(0)
    ins = {"x": rng.standard_normal((B, S, H)).astype(np.float32),
           "y": rng.standard_normal((B, S, H)).astype(np.float32)}
    for w in W768 + W1536 + ["dq_w", "dk_w"]:
        shp = (H, H) if w not in W1536 else (2 * H, H)
        ins[w] = (rng.standard_normal(shp) * 0.02).astype(np.float32)
    ins["gate_w"] = (rng.standard_normal((2 * H, 1)) * 0.02).astype(np.float32)
    ins["nf_out_w"] = (rng.standard_normal((2 * H, 1)) * 0.02).astype(np.float32)
    for b in BIAS + ["vv_b", "dq_b", "dk_b"]:
        ins[b] = np.zeros(H, np.float32)
    out = kernel(**ins)
    print("out", out.shape, out.dtype, np.abs(out).mean())


# revision 20
# speedup vs baseline: 1.1900x; 1.0321x over previous
"""Coupled-attention module as a distributed Bass/Tile kernel on 8 TRN2 cores.

Math notes (exact algebra, not approximations):
- The differential-attention scores are constant along the softmax axis, so
  softmax yields exactly uniform 1/S weights: diff_vector collapses to the
  per-batch mean of (y @ dv_w + dv_b), broadcast over sequence. dq/dk are dead.
- Sharding: rows of the flattened (B*S, H) activations, 256 per core; cores
  0-3 own batch 0, 4-7 batch 1. Each core redundantly computes full-batch K/V
  (collective reshards measure slower than the redundant GEMMs on this part).
- Attention head pairs are packed into disjoint PE row groups (K=64 each), so
  the two scores matmuls of a pair run concurrently, and both heads' scores
  share one [128,512] PSUM bank -> one big exp ACTIVATE per (pair, block).
- The two sequence-axis softmax denominators in the gating network are summed
  across the 4-core batch group with small AllGathers + local adds (lower
  floor than AllReduce).
- Per-batch constants (m, th1, bias1, bias2) use vector-stationary matmuls
  (weights as the moving operand) with DRAM-bounce transposes, emitted inside
  the attention phase so they fill TensorE gaps while ACT computes exp.
- Compute in bf16 with fp32 accumulation; exp/tanh/sigmoid on ACT; identity
  epilogues on DVE to keep ACT for transcendentals.
"""

import numpy as np
import ml_dtypes

import concourse.bass as bass
import concourse.mybir as mybir
import concourse.tile as tile
from concourse import bacc
from concourse.bass_utils import run_bass_kernel_spmd

B, S, H = 2, 1024, 768
NH, DH = 12, 64
P = 128
RV = 256            # rows per core
KC = H // P         # 6 channel chunks
JC = S // P         # 8 sequence chunks
GROUPS = [[0, 1, 2, 3], [4, 5, 6, 7]]
SCALE = 1.0 / 8.0   # 1/sqrt(DH)

bf16 = mybir.dt.bfloat16
f32 = mybir.dt.float32
AF = mybir.ActivationFunctionType
ALU = mybir.AluOpType
nbf16 = ml_dtypes.bfloat16

W768 = ["vq_w", "vk_w", "vv_w", "dv_w", "WD_w", "van_fc_w", "WV_w", "diff_fc_w",
        "diff_fus_w", "van_fus_w", "nf_w", "final_w"]
W1536 = ["d_theta_w", "v_gamma_w", "diff_out_w", "van_out_w"]
BIAS = ["vq_b", "vk_b", "dv_b", "van_fc_b", "d_theta_b", "diff_fc_b",
        "v_gamma_b", "diff_out_b", "van_out_b", "diff_fus_b", "van_fus_b",
        "nf_b", "final_b"]


def build(has_vvb: bool):
    nc = bacc.Bacc(None, target_bir_lowering=False, debug=False, num_devices=8)

    xT_d = nc.dram_tensor("xT", [H, RV], bf16, kind="ExternalInput")
    yT_d = nc.dram_tensor("yT", [H, S], bf16, kind="ExternalInput")
    wd = {}
    for w in W768:
        wd[w] = nc.dram_tensor(w, [H, H], bf16, kind="ExternalInput")
    for w in W1536:
        wd[w] = nc.dram_tensor(w, [2 * H, H], bf16, kind="ExternalInput")
    wd["gate_w"] = nc.dram_tensor("gate_w", [2 * H, 1], bf16, kind="ExternalInput")
    wd["nf_out_w"] = nc.dram_tensor("nf_out_w", [2 * H, 1], bf16, kind="ExternalInput")
    bd = {}
    for b in BIAS:
        bd[b] = nc.dram_tensor(b, [H], f32, kind="ExternalInput")
    if has_vvb:
        bd["vv_b"] = nc.dram_tensor("vv_b", [H], f32, kind="ExternalInput")
    out_d = nc.dram_tensor("outT", [H, RV], f32, kind="ExternalOutput")

    with tile.TileContext(nc, num_cores=8) as tc:
        with (
            tc.tile_pool(name="wpool", bufs=4) as wp,
            tc.tile_pool(name="wsmall", bufs=2) as wsp,
            tc.tile_pool(name="acts", bufs=1) as ap,
            tc.tile_pool(name="loop", bufs=2) as lp,
            tc.tile_pool(name="psum", bufs=8, space="PSUM") as pp,
            tc.tile_pool(name="dram", bufs=4, space="DRAM") as dp,
        ):
            def wtile(name, half=None):
                t = wp.tile([P, KC, H], bf16, name=f"w_{name}_{half}", tag="w")
                src = wd[name]
                if half is not None:
                    src = src[half * H:(half + 1) * H, :]
                src = src.rearrange("(kc p) n -> kc p n", p=P)
                for kc in range(KC):
                    nc.sync.dma_start(t[:, kc, :], src[kc])
                return t

            def btile(name):
                t = ap.tile([P, KC], f32, name=f"b_{name}")
                nc.sync.dma_start(t[:], bd[name].rearrange("(c p) -> p c", p=P))
                return t

            def brow(name):
                t = ap.tile([1, H], f32, name=f"br_{name}")
                nc.sync.dma_start(t[:], bd[name].rearrange("(o c) -> o c", o=1))
                return t

            # ---------------- Q projection first: minimal-dependency PE work
            b_vq = btile("vq_b")
            xT = ap.tile([P, KC, RV], bf16, name="xT")
            for kc in range(KC):
                nc.sync.dma_start(xT[:, kc, :], xT_d.rearrange(
                    "(kc p) n -> kc p n", p=P)[kc])
            w_vq = wtile("vq_w")
            b_vk = btile("vk_b")
            w_vk = wtile("vk_w")
            yT = ap.tile([P, KC, S], bf16, name="yT")
            for kc in range(KC):
                nc.sync.dma_start(yT[:, kc, :], yT_d.rearrange(
                    "(kc p) n -> kc p n", p=P)[kc])
            w_vv = wtile("vv_w")

            qT = ap.tile([P, KC, RV], bf16, name="qT")
            for mc in range(KC):
                ps = pp.tile([P, RV], f32, name=f"qps{mc}", tag="big", bufs=3)
                for kc in range(KC):
                    nc.tensor.matmul(ps[:], w_vq[:, kc, mc * P:(mc + 1) * P],
                                     xT[:, kc, :],
                                     start=(kc == 0), stop=(kc == KC - 1))
                nc.vector.tensor_scalar_add(qT[:, mc, :], ps[:],
                                            b_vq[:, mc:mc + 1])

            ones64 = ap.tile([1, 64], bf16, name="ones64")
            nc.vector.memset(ones64[:], 1.0)
            ones128 = ap.tile([1, P], bf16, name="ones128")
            nc.vector.memset(ones128[:], 1.0)

            # warm up the collective stream early: the first real collective
            # otherwise pays an ~11.7us trigger-start delay.
            dgi = dp.tile([P, 1], f32, name="dgi")
            dgo = dp.tile([4 * P, 1], f32, name="dgo")
            nc.sync.dma_start(dgi[:], b_vq[:, 0:1])
            nc.gpsimd.collective_compute(
                "AllGather", ALU.bypass, replica_groups=GROUPS,
                ins=[dgi[:]], outs=[dgo[:]])

            # ---------------- K/V projections, emitted in two halves so the
            # attention pairs of the first half overlap the second half ------
            kT = ap.tile([P, KC, S], bf16, name="kT")
            v_aug = ap.tile([P, JC, NH, DH + 1], bf16, name="v_aug")
            nc.vector.memset(v_aug[:, :, :, DH:DH + 1], 1.0)

            def kproj(mc):
                for sh in range(2):
                    ps = pp.tile([P, 512], f32, name=f"kps{mc}_{sh}",
                                 tag="big", bufs=3)
                    for kc in range(KC):
                        nc.tensor.matmul(
                            ps[:], w_vk[:, kc, mc * P:(mc + 1) * P],
                            yT[:, kc, sh * 512:(sh + 1) * 512],
                            start=(kc == 0), stop=(kc == KC - 1))
                    nc.vector.tensor_scalar_add(
                        kT[:, mc, sh * 512:(sh + 1) * 512], ps[:],
                        b_vk[:, mc:mc + 1])

            def vproj(cg):
                for jc in range(JC):
                    ps = pp.tile([P, 384], f32, name=f"vps{jc}_{cg}",
                                 tag="big", bufs=3)
                    for kc in range(KC):
                        nc.tensor.matmul(
                            ps[:], yT[:, kc, jc * P:(jc + 1) * P],
                            w_vv[:, kc, cg * 384:(cg + 1) * 384],
                            start=(kc == 0), stop=(kc == KC - 1))
                    nc.vector.tensor_copy(
                        v_aug[:, jc, cg * 6:(cg + 1) * 6, 0:DH],
                        ps[:].rearrange("p (h d) -> p h d", d=DH))

            # ---------------- per-batch chain pieces ------------------------
            yb = ap.tile([P, KC], f32, name="yb")
            for kc in range(KC):
                nc.vector.tensor_reduce(yb[:, kc:kc + 1], yT[:, kc, :],
                                        axis=mybir.AxisListType.X, op=ALU.add)
            ybt = ap.tile([P, KC], bf16, name="ybt")
            nc.vector.tensor_scalar_mul(ybt[:], yb[:], 1.0 / S)

            def vchain(vec_cm, w_t, func, bias_row, name):
                # row-major out [1, 768] = func(vec @ W + bias); vec chan-major
                # [128, 6] bf16 is the stationary operand (weights stream).
                pr = []
                for half in range(2):
                    ps = pp.tile([1, 384], f32, name=f"{name}ps{half}",
                                 tag="sps", bufs=2)
                    for kc in range(KC):
                        nc.tensor.matmul(ps[:], vec_cm[:, kc:kc + 1],
                                         w_t[:, kc, half * 384:(half + 1) * 384],
                                         start=(kc == 0), stop=(kc == KC - 1))
                    pr.append(ps)
                out = ap.tile([1, H], f32, name=f"{name}_row")
                for half in range(2):
                    osl = out[:, half * 384:(half + 1) * 384]
                    bsl = (None if bias_row is None
                           else bias_row[:, half * 384:(half + 1) * 384])
                    if func == AF.Identity:
                        if bsl is not None:
                            nc.vector.tensor_add(osl, pr[half][:], bsl)
                        else:
                            nc.vector.tensor_copy(osl, pr[half][:])
                    else:
                        src2 = pr[half]
                        if bsl is not None:
                            tmp = lp.tile([1, 384], f32, name=f"{name}tmp{half}",
                                          tag="chtmp")
                            nc.vector.tensor_add(tmp[:], src2[:], bsl)
                            src2 = tmp
                        nc.scalar.activation(osl, src2[:], func)
                return out

            def to_chan(row_t, dt, name):
                # [1, 768] row vector -> chan-major [128, 6] via DRAM bounce
                db = dp.tile([H], dt, name=f"db_{name}")
                if dt == f32:
                    nc.sync.dma_start(db.rearrange("(o c) -> o c", o=1), row_t[:])
                else:
                    cast = lp.tile([1, H], dt, name=f"cast_{name}", tag="chcast")
                    nc.vector.tensor_copy(cast[:], row_t[:])
                    nc.sync.dma_start(db.rearrange("(o c) -> o c", o=1), cast[:])
                out = ap.tile([P, KC], dt, name=f"cm_{name}")
                nc.sync.dma_start(out[:], db.rearrange("(c p) -> p c", p=P))
                return out

            # ---------------- attention: pairs pipelined against K/V -------
            w_dv = wtile("dv_w")
            b_dv_row = brow("dv_b")
            w_WD = wtile("WD_w")
            w_dth0 = wtile("d_theta_w", half=0)
            b_dth_row = brow("d_theta_b")
            w_dout0 = wtile("diff_out_w", half=0)
            b_dout_row = brow("diff_out_b")
            if has_vvb:
                b_vv = btile("vv_b")
            vanT = ap.tile([P, KC, RV], bf16, name="vanT")

            def head_tail(h, pv):
                hp, hh = h // 2, h % 2
                invZb = lp.tile([1, RV], bf16, name=f"invZb{h}", tag="invZb")
                with nc.allow_low_precision(reason="softmax 1/Z feeds bf16 mul"):
                    nc.vector.reciprocal(invZb[:], pv[DH:DH + 1, :])
                bc = pp.tile([64, RV], f32, name=f"bc{h}", tag="big", bufs=3)
                nc.tensor.matmul(bc[:], ones64[:], invZb[:],
                                 start=True, stop=True)
                bcs = lp.tile([64, RV], bf16, name=f"bcs{h}", tag="bcs")
                nc.vector.tensor_copy(bcs[:], bc[:])
                nc.vector.tensor_mul(vanT[hh * 64:hh * 64 + 64, hp, :],
                                     pv[0:DH, :], bcs[:])
                if has_vvb:
                    nc.vector.tensor_scalar_add(
                        vanT[hh * 64:hh * 64 + 64, hp, :],
                        vanT[hh * 64:hh * 64 + 64, hp, :],
                        b_vv[hh * 64:hh * 64 + 64, hp:hp + 1])

            def pair_block(hp, prev):
                # scores+exp for pair hp, with the PV matmuls of the previous
                # pair interleaved into the same jc loop so the PE never
                # stalls on ACT's exp backlog.
                e = lp.tile([P, JC, 512], bf16, name=f"e{hp}", tag="expT",
                            bufs=3)
                if prev is not None:
                    hq, eq = prev
                    pvs = [pp.tile([DH + 1, RV], f32, name=f"pv{2 * hq + hh}",
                                   tag="pv", bufs=3) for hh in range(2)]
                for jc in range(JC):
                    for hh in range(2):
                        lo = hh * 64
                        sc = pp.tile([P, RV], f32, name=f"sc{hp}_{jc}_{hh}",
                                     tag="big", bufs=3)
                        nc.tensor.matmul(
                            sc[:],
                            kT[lo:lo + 64, hp, jc * P:(jc + 1) * P],
                            qT[lo:lo + 64, hp, :],
                            start=True, stop=True)
                        nc.scalar.activation(
                            e[:, jc, hh * RV:(hh + 1) * RV], sc[:],
                            AF.Exp, scale=SCALE)
                    if prev is not None:
                        for hh in range(2):
                            nc.tensor.matmul(
                                pvs[hh][:], v_aug[:, jc, 2 * hq + hh, :],
                                eq[:, jc, hh * RV:(hh + 1) * RV],
                                start=(jc == 0), stop=(jc == JC - 1))
                if prev is not None:
                    for hh in range(2):
                        head_tail(2 * hq + hh, pvs[hh])
                return e

            def last_pv(hq, eq):
                pvs = [pp.tile([DH + 1, RV], f32, name=f"pv{2 * hq + hh}",
                               tag="pv", bufs=3) for hh in range(2)]
                for jc in range(JC):
                    for hh in range(2):
                        nc.tensor.matmul(
                            pvs[hh][:], v_aug[:, jc, 2 * hq + hh, :],
                            eq[:, jc, hh * RV:(hh + 1) * RV],
                            start=(jc == 0), stop=(jc == JC - 1))
                for hh in range(2):
                    head_tail(2 * hq + hh, pvs[hh])

            kproj(0)
            kproj(1)
            kproj(2)
            vproj(0)
            e0 = pair_block(0, None)
            m_row = vchain(ybt, w_dv, AF.Identity, b_dv_row, "m")
            e1h = pair_block(1, (0, e0))
            m32 = to_chan(m_row, f32, "m32")
            m_cm = ap.tile([P, KC], bf16, name="m_cm")
            nc.vector.tensor_copy(m_cm[:], m32[:])
            e2h = pair_block(2, (1, e1h))
            kproj(3)
            kproj(4)
            kproj(5)
            vproj(1)
            th1_row = vchain(m_cm, w_WD, AF.Tanh, None, "th1")
            e3h = pair_block(3, (2, e2h))
            th1_cm = to_chan(th1_row, bf16, "th1")
            bias1_row = vchain(th1_cm, w_dth0, AF.Identity, b_dth_row, "bias1")
            e4h = pair_block(4, (3, e3h))
            bias2_row = vchain(m_cm, w_dout0, AF.Identity, b_dout_row, "bias2")
            e5h = pair_block(5, (4, e4h))
            bias1 = to_chan(bias1_row, f32, "bias1")
            bias2 = to_chan(bias2_row, f32, "bias2")
            last_pv(5, e5h)

            # ---------------- gating network --------------------------------
            def gemm(pairs, func, bias_t=None, accum_t=None, name="g",
                     out_dt=bf16, pre=None):
                out = ap.tile([P, KC, RV], out_dt, name=name)
                nmm = len(pairs) * KC
                for mc in range(KC):
                    ps = pp.tile([P, RV], f32, name=f"{name}ps{mc}", tag="big",
                                 bufs=3)
                    i = 0
                    for wt, at in pairs:
                        for kc in range(KC):
                            nc.tensor.matmul(ps[:],
                                             wt[:, kc, mc * P:(mc + 1) * P],
                                             at[:, kc, :],
                                             start=(i == 0), stop=(i == nmm - 1))
                            i += 1
                    src = ps
                    if pre is not None:
                        tmp = lp.tile([P, RV], f32, name=f"{name}pre{mc}",
                                      tag="pretmp")
                        nc.vector.tensor_add(tmp[:], ps[:], pre[:, mc, :])
                        src = tmp
                    if func == AF.Identity and accum_t is None:
                        if bias_t is not None:
                            nc.vector.tensor_scalar_add(out[:, mc, :], src[:],
                                                        bias_t[:, mc:mc + 1])
                        else:
                            nc.vector.tensor_copy(out[:, mc, :], src[:])
                    else:
                        nc.scalar.activation(
                            out[:, mc, :], src[:], func,
                            bias=(bias_t[:, mc:mc + 1] if bias_t is not None
                                  else 0.0),
                            accum_out=(accum_t[:, mc:mc + 1]
                                       if accum_t is not None else None))
                return out

            def allgather6(part, name):
                gi = dp.tile([P, KC], f32, name=f"gi_{name}")
                go = dp.tile([4 * P, KC], f32, name=f"go_{name}")
                nc.sync.dma_start(gi[:], part[:])
                nc.gpsimd.collective_compute(
                    "AllGather", ALU.bypass, replica_groups=GROUPS,
                    ins=[gi[:]], outs=[go[:]])
                zt = ap.tile([P, 4, KC], f32, name=f"zt_{name}")
                nc.sync.dma_start(zt[:], go.rearrange("(r p) c -> p r c", p=P))
                z = ap.tile([P, KC], f32, name=f"z_{name}")
                nc.vector.tensor_add(z[:], zt[:, 0, :], zt[:, 1, :])
                nc.vector.tensor_add(z[:], z[:], zt[:, 2, :])
                nc.vector.tensor_add(z[:], z[:], zt[:, 3, :])
                return z

            w_vfc = wtile("van_fc_w")
            b_vfc = btile("van_fc_b")
            theta2 = gemm([(w_vfc, vanT)], AF.Tanh, bias_t=b_vfc, name="theta2")

            w_dth1 = wtile("d_theta_w", half=1)
            part1 = ap.tile([P, KC], f32, name="part1")
            e1 = gemm([(w_dth1, theta2)], AF.Exp, bias_t=bias1, accum_t=part1,
                      name="e1")
            z1 = allgather6(part1, "z1")

            # --- AllGather-1 bubble fillers (independent of z1) -------------
            w_WV = wtile("WV_w")
            gamma1 = gemm([(w_WV, vanT)], AF.Tanh, name="gamma1")
            w_vg0 = wtile("v_gamma_w", half=0)
            b_vg = btile("v_gamma_b")
            z2a = gemm([(w_vg0, gamma1)], AF.Identity, bias_t=b_vg, name="z2a",
                       out_dt=f32)
            w_vo0 = wtile("van_out_w", half=0)
            b_vo = btile("van_out_b")
            voa = gemm([(w_vo0, vanT)], AF.Identity, bias_t=b_vo, name="voa",
                       out_dt=f32)

            s1 = ap.tile([P, KC], f32, name="s1")
            nc.vector.reciprocal(s1[:], z1[:])
            nc.vector.tensor_mul(s1[:], s1[:], m32[:])
            dth = ap.tile([P, KC, RV], bf16, name="dth")
            for mc in range(KC):
                nc.vector.tensor_scalar_mul(dth[:, mc, :], e1[:, mc, :],
                                            s1[:, mc:mc + 1])

            w_dfc = wtile("diff_fc_w")
            b_dfc = btile("diff_fc_b")
            gamma2 = gemm([(w_dfc, dth)], AF.Tanh, bias_t=b_dfc, name="gamma2")

            w_vg1 = wtile("v_gamma_w", half=1)
            part2 = ap.tile([P, KC], f32, name="part2")
            e2 = gemm([(w_vg1, gamma2)], AF.Exp, accum_t=part2, pre=z2a,
                      name="e2")
            z2 = allgather6(part2, "z2")

            # --- AllGather-2 bubble fillers (diff branch tail) --------------
            w_dout1 = wtile("diff_out_w", half=1)
            dout = gemm([(w_dout1, dth)], AF.Tanh, bias_t=bias2, name="dout")
            w_dfus = wtile("diff_fus_w")
            b_dfus = btile("diff_fus_b")
            dfus = gemm([(w_dfus, dout)], AF.Tanh, bias_t=b_dfus, name="dfus")

            s2 = ap.tile([P, KC], f32, name="s2")
            nc.vector.reciprocal(s2[:], z2[:])
            ag = ap.tile([P, KC, RV], bf16, name="ag")
            for mc in range(KC):
                nc.vector.scalar_tensor_tensor(
                    ag[:, mc, :], e2[:, mc, :], s2[:, mc:mc + 1],
                    vanT[:, mc, :], op0=ALU.mult, op1=ALU.mult)

            w_vo1 = wtile("van_out_w", half=1)
            vout = gemm([(w_vo1, ag)], AF.Tanh, pre=voa, name="vout")
            w_vfus = wtile("van_fus_w")
            b_vfus = btile("van_fus_b")
            vfus = gemm([(w_vfus, vout)], AF.Tanh, bias_t=b_vfus, name="vfus")

            # gate (M=1 GEMM over both fusion tensors)
            def vec_unit(wname, act_pairs, name):
                wt = wsp.tile([P, 2 * KC, 1], bf16, name=f"ws_{name}", tag="ws")
                nc.sync.dma_start(wt[:], wd[wname].rearrange(
                    "(c p) o -> p c o", p=P))
                ps = pp.tile([1, RV], f32, name=f"{name}ps", tag="sps", bufs=2)
                i = 0
                for at, base in act_pairs:
                    for kc in range(KC):
                        nc.tensor.matmul(ps[:], wt[:, base + kc, :],
                                         at[:, kc, :],
                                         start=(i == 0), stop=(i == 2 * KC - 1))
                        i += 1
                out = ap.tile([1, RV], f32, name=f"v_{name}")
                nc.scalar.activation(out[:], ps[:], AF.Sigmoid)
                return out

            g = vec_unit("gate_w", [(dfus, 0), (vfus, KC)], "gate")
            gb16 = ap.tile([1, RV], bf16, name="gb16")
            nc.vector.tensor_copy(gb16[:], g[:])
            gbc = pp.tile([P, RV], f32, name="gbc", tag="pv", bufs=3)
            nc.tensor.matmul(gbc[:], ones128[:], gb16[:], start=True, stop=True)

            fus = ap.tile([P, KC, RV], bf16, name="fus")
            for mc in range(KC):
                t1 = lp.tile([P, RV], bf16, name=f"ft1_{mc}", tag="ft1")
                nc.vector.tensor_sub(t1[:], vfus[:, mc, :], dfus[:, mc, :])
                t2 = lp.tile([P, RV], bf16, name=f"ft2_{mc}", tag="ft2")
                nc.vector.tensor_mul(t2[:], t1[:], gbc[:])
                nc.vector.tensor_add(fus[:, mc, :], t2[:], dfus[:, mc, :])

            w_nf = wtile("nf_w")
            b_nf = btile("nf_b")
            tnf = gemm([(w_nf, fus)], AF.Identity, bias_t=b_nf, name="tnf")
            w_fin = wtile("final_w")
            b_fin = btile("final_b")
            ft = gemm([(w_fin, fus)], AF.Tanh, bias_t=b_fin, name="ftanh")
            nfv = vec_unit("nf_out_w", [(vanT, 0), (tnf, KC)], "nf")
            nb16 = ap.tile([1, RV], bf16, name="nb16")
            nc.vector.tensor_copy(nb16[:], nfv[:])
            nbc = pp.tile([P, RV], f32, name="nbc", tag="pv", bufs=3)
            nc.tensor.matmul(nbc[:], ones128[:], nb16[:], start=True, stop=True)
            od = out_d.rearrange("(mc p) n -> mc p n", p=P)
            for mc in range(KC):
                ot = lp.tile([P, RV], f32, name=f"ot{mc}", tag="ot")
                nc.vector.tensor_mul(ot[:], ft[:, mc, :], nbc[:])
                nc.sync.dma_start(od[mc], ot[:])

    nc.compile()
    return nc


_CACHE = {}


def _prep_in_maps(inputs):
    x = np.asarray(inputs["x"], np.float32)
    y = np.asarray(inputs["y"], np.float32)
    has_vvb = bool(np.any(np.asarray(inputs["vv_b"]) != 0))

    xt = np.ascontiguousarray(x.reshape(B * S, H).T).astype(nbf16)   # [H, 2048]
    yts = [np.ascontiguousarray(y[b].T).astype(nbf16) for b in range(B)]

    base = {}
    for w in W768 + W1536 + ["gate_w", "nf_out_w"]:
        base[w] = np.asarray(inputs[w], np.float32).astype(nbf16)
    for b in BIAS:
        base[b] = np.ascontiguousarray(np.asarray(inputs[b], np.float32))
    if has_vvb:
        base["vv_b"] = np.ascontiguousarray(np.asarray(inputs["vv_b"], np.float32))

    in_maps = []
    for c in range(8):
        bat = c // 4
        m = dict(base)
        m["xT"] = np.ascontiguousarray(xt[:, c * RV:(c + 1) * RV])
        m["yT"] = yts[bat]
        in_maps.append(m)
    return in_maps, has_vvb


def kernel(**inputs):
    in_maps, has_vvb = _prep_in_maps(inputs)
    if has_vvb not in _CACHE:
        _CACHE[has_vvb] = build(has_vvb)
    nc = _CACHE[has_vvb]

    res = run_bass_kernel_spmd(nc, in_maps, core_ids=list(range(8)))
    full = np.concatenate([res.results[c]["outT"] for c in range(8)], axis=1)
    return np.ascontiguousarray(full.T.reshape(B, S, H)).astype(np.float32)


if __name__ == "__main__":
    rng = np.random.default_rng(0)
    ins = {"x": rng.standard_normal((B, S, H)).astype(np.float32),
           "y": rng.standard_normal((B, S, H)).astype(np.float32)}
    for w in W768 + W1536 + ["dq_w", "dk_w"]:
        shp = (H, H) if w not in W1536 else (2 * H, H)
        ins[w] = (rng.standard_normal(shp) * 0.02).astype(np.float32)
    ins["gate_w"] = (rng.standard_normal((2 * H, 1)) * 0.02).astype(np.float32)
    ins["nf_out_w"] = (rng.standard_normal((2 * H, 1)) * 0.02).astype(np.float32)
    for b in BIAS + ["vv_b", "dq_b", "dk_b"]:
        ins[b] = np.zeros(H, np.float32)
    out = kernel(**ins)
    print("out", out.shape, out.dtype, np.abs(out).mean())
